# revision 11
# baseline (speedup 1.0000x reference)
"""Trainium2 Bass kernel for the pooling+MLP model (nn_BaseModel_79250736546631).

Computation (per batch row b):
    mask  = (x[b, :200] > 0)
    avg   = mean(mask)                      # count/200, a per-row scalar
    user_vec = sum_h mask[h]*avg * emb[x[b,h]]
             = avg * (sum_h emb[x[b,h]] - n_zero * emb[0])
    h = concat(user_vec, emb[x[b,200]])
    out = sigmoid(relu(relu(h@W1+b1)@W2+b2)@W3+b3)

Sharding: data-parallel over batch across 8 NeuronCores; the 1M x 64
embedding table and the tiny MLP are replicated per core.

Per core (2048 rows): for each 128-row tile, one indirect DMA gathers all
128*201 embedding rows into SBUF [128, 201*64]; a strided DVE reduce sums
the 200 history slots; the mask/avg correction is applied with per-partition
scalars; a PE transpose puts [user_vec | ad_emb] feature-major and the MLP
runs as three lhsT=W matmuls with fused bias+activation on the ACT engine.
"""

import sys

for _p in ("/opt/trn_rl_repo",):
    if _p not in sys.path:
        sys.path.insert(0, _p)

import numpy as np

P = 128
EMB = 64
HIST = 200
NIDX = HIST + 1
VOCAB = 1_000_000
B = 16384
NCORES = 8
B_CORE = B // NCORES  # 2048


def build_nc(vocab=VOCAB, b_core=B_CORE, debug_outs=False, reps=1):
    """Build + compile the per-core Bass program (SPMD, same on all cores)."""
    import concourse.bacc as bacc
    import concourse.bass as bass
    import concourse.mybir as mybir
    import concourse.tile as tile
    from concourse.masks import make_identity

    f32 = mybir.dt.float32
    i32 = mybir.dt.int32
    tiles = b_core // P

    nc = bacc.Bacc("TRN2", target_bir_lowering=False, debug=False)

    x_d = nc.dram_tensor("x", [b_core, NIDX], i32, kind="ExternalInput").ap()
    emb_d = nc.dram_tensor("emb", [vocab, EMB], f32, kind="ExternalInput").ap()
    w1_d = nc.dram_tensor("W1", [2 * EMB, 120], f32, kind="ExternalInput").ap()
    b1_d = nc.dram_tensor("b1", [120], f32, kind="ExternalInput").ap()
    w2_d = nc.dram_tensor("W2", [120, 60], f32, kind="ExternalInput").ap()
    b2_d = nc.dram_tensor("b2", [60], f32, kind="ExternalInput").ap()
    w3_d = nc.dram_tensor("W3", [60, 1], f32, kind="ExternalInput").ap()
    b3_d = nc.dram_tensor("b3", [1], f32, kind="ExternalInput").ap()
    out_d = nc.dram_tensor("out", [tiles, P], f32, kind="ExternalOutput").ap()
    if debug_outs:
        dbg_s = nc.dram_tensor("dbg_s", [b_core, EMB], f32, kind="ExternalOutput").ap()
        dbg_cnt = nc.dram_tensor("dbg_cnt", [b_core, 1], f32, kind="ExternalOutput").ap()
        dbg_h = nc.dram_tensor("dbg_h", [b_core, 2 * EMB], f32, kind="ExternalOutput").ap()
        dbg_ht = nc.dram_tensor("dbg_ht", [tiles, P, P], f32, kind="ExternalOutput").ap()
        dbg_g = nc.dram_tensor("dbg_g", [P, NIDX * EMB], f32, kind="ExternalOutput").ap()

    with tile.TileContext(nc) as tc:
        with (
            tc.tile_pool(name="const", bufs=1) as cpool,
            tc.tile_pool(name="xin", bufs=3) as xpool,
            tc.tile_pool(name="gather", bufs=2) as gpool,
            tc.tile_pool(name="small", bufs=3) as spool,
            tc.tile_pool(name="psum", bufs=2, space="PSUM") as ppool,
        ):
            # ---- constants / weights (loaded once) ----
            w1_s = cpool.tile([2 * EMB, 120], f32)
            nc.sync.dma_start(out=w1_s[:], in_=w1_d[:])
            w2_s = cpool.tile([120, 60], f32)
            nc.sync.dma_start(out=w2_s[:], in_=w2_d[:])
            w3_s = cpool.tile([60, 1], f32)
            nc.sync.dma_start(out=w3_s[:], in_=w3_d[:])
            b1_s = cpool.tile([120, 1], f32)
            nc.sync.dma_start(out=b1_s[:], in_=b1_d[:, None])
            b2_s = cpool.tile([60, 1], f32)
            nc.sync.dma_start(out=b2_s[:], in_=b2_d[:, None])
            b3_s = cpool.tile([1, 1], f32)
            nc.sync.dma_start(out=b3_s[:], in_=b3_d[:, None])

            identity = cpool.tile([P, P], f32)
            make_identity(nc, identity[:])

            # emb[0] broadcast to all 128 partitions via a rank-1 matmul
            emb0_s = cpool.tile([1, EMB], f32)
            nc.sync.dma_start(out=emb0_s[:], in_=emb_d[0:1, :])
            ones1 = cpool.tile([1, P], f32)
            nc.vector.memset(ones1[:], 1.0)
            emb0_p = ppool.tile([P, EMB], f32, space="PSUM", tag="htp")
            nc.tensor.matmul(
                out=emb0_p[:], lhsT=ones1[:], rhs=emb0_s[:], start=True, stop=True
            )
            emb0_b = cpool.tile([P, EMB], f32)
            nc.vector.tensor_copy(out=emb0_b[:], in_=emb0_p[:])

            # ---- per-tile pipeline ----
            for t in range(tiles * reps):
                t = t % tiles
                rows = slice(t * P, (t + 1) * P)

                x_t = xpool.tile([P, NIDX], i32, tag="x")
                nc.sync.dma_start(out=x_t[:], in_=x_d[rows, :])

                # gather all 201 embedding rows per batch row: [128, 201*64].
                # HW indirect DMA consumes one index per dst partition row, so
                # each history slot needs its own instruction.
                g_t = gpool.tile([P, NIDX * EMB], f32, tag="g")
                for h in range(NIDX):
                    nc.gpsimd.indirect_dma_start(
                        out=g_t[:, h * EMB : (h + 1) * EMB],
                        out_offset=None,
                        in_=emb_d[:],
                        in_offset=bass.IndirectOffsetOnAxis(
                            ap=x_t[:, h : h + 1], axis=0
                        ),
                    )

                # count of nonzero history ids (cast -> is_gt 1.0/0.0, accumulated)
                xf_t = spool.tile([P, HIST], f32, tag="xf")
                nc.vector.tensor_copy(out=xf_t[:], in_=x_t[:, :HIST])
                mask_t = spool.tile([P, HIST], f32, tag="mask")
                cnt_t = spool.tile([P, 1], f32, tag="cnt")
                nc.vector.tensor_scalar(
                    out=mask_t[:],
                    in0=xf_t[:],
                    scalar1=0.0,
                    scalar2=None,
                    op0=mybir.AluOpType.is_gt,
                    op1=mybir.AluOpType.add,
                    accum_out=cnt_t[:],
                )
                # avg = cnt/200 ; n0 = 200-cnt ; c = avg*n0
                avg_t = spool.tile([P, 1], f32, tag="avg")
                nc.scalar.mul(avg_t[:], cnt_t[:], 1.0 / HIST)
                n0_t = spool.tile([P, 1], f32, tag="n0")
                nc.vector.tensor_scalar(
                    out=n0_t[:],
                    in0=cnt_t[:],
                    scalar1=float(HIST),
                    scalar2=-1.0,
                    op0=mybir.AluOpType.subtract,
                    op1=mybir.AluOpType.mult,
                )
                c_t = spool.tile([P, 1], f32, tag="c")
                nc.vector.tensor_tensor(
                    out=c_t[:], in0=avg_t[:], in1=n0_t[:], op=mybir.AluOpType.mult
                )

                # S = sum over the 200 history slots (strided view [p, j, h])
                g_v = g_t[:].rearrange("p (h j) -> p j h", j=EMB)
                s_t = spool.tile([P, EMB], f32, tag="s")
                nc.vector.reduce_sum(
                    out=s_t[:], in_=g_v[:, :, :HIST], axis=mybir.AxisListType.X
                )

                # H = [avg*S - c*emb0 | ad_emb]
                h_t = spool.tile([P, 2 * EMB], f32, tag="h")
                nc.vector.tensor_scalar_mul(
                    out=h_t[:, :EMB], in0=s_t[:], scalar1=avg_t[:, 0:1]
                )
                e0c_t = spool.tile([P, EMB], f32, tag="e0c")
                nc.vector.tensor_scalar_mul(
                    out=e0c_t[:], in0=emb0_b[:], scalar1=c_t[:, 0:1]
                )
                nc.vector.tensor_tensor(
                    out=h_t[:, :EMB],
                    in0=h_t[:, :EMB],
                    in1=e0c_t[:],
                    op=mybir.AluOpType.subtract,
                )
                nc.vector.tensor_copy(
                    out=h_t[:, EMB:], in_=g_t[:, HIST * EMB : NIDX * EMB]
                )

                # transpose to feature-major [128 features, 128 batch]
                ht_p = ppool.tile([P, P], f32, space="PSUM", tag="htp")
                nc.tensor.transpose(out=ht_p[:], in_=h_t[:], identity=identity[:])
                ht_s = spool.tile([P, P], f32, tag="hts")
                nc.vector.tensor_copy(out=ht_s[:], in_=ht_p[:])

                # MLP: z = W.T @ a, bias+activation fused on ACT
                z1_p = ppool.tile([120, P], f32, space="PSUM", tag="z1")
                nc.tensor.matmul(
                    out=z1_p[:], lhsT=w1_s[:], rhs=ht_s[:], start=True, stop=True
                )
                a1_s = spool.tile([120, P], f32, tag="a1")
                nc.scalar.activation(
                    out=a1_s[:],
                    in_=z1_p[:],
                    func=mybir.ActivationFunctionType.Relu,
                    bias=b1_s[:, 0:1],
                )

                z2_p = ppool.tile([60, P], f32, space="PSUM", tag="z2")
                nc.tensor.matmul(
                    out=z2_p[:], lhsT=w2_s[:], rhs=a1_s[:], start=True, stop=True
                )
                a2_s = spool.tile([60, P], f32, tag="a2")
                nc.scalar.activation(
                    out=a2_s[:],
                    in_=z2_p[:],
                    func=mybir.ActivationFunctionType.Relu,
                    bias=b2_s[:, 0:1],
                )

                z3_p = ppool.tile([1, P], f32, space="PSUM", tag="z2")
                nc.tensor.matmul(
                    out=z3_p[:], lhsT=w3_s[:], rhs=a2_s[:], start=True, stop=True
                )
                o_s = spool.tile([1, P], f32, tag="o")
                nc.scalar.activation(
                    out=o_s[:],
                    in_=z3_p[:],
                    func=mybir.ActivationFunctionType.Sigmoid,
                    bias=b3_s[:, 0:1],
                )

                nc.sync.dma_start(out=out_d[t : t + 1, :], in_=o_s[:])

                if debug_outs:
                    nc.sync.dma_start(out=dbg_s[rows, :], in_=s_t[:])
                    nc.sync.dma_start(out=dbg_cnt[rows, :], in_=cnt_t[:])
                    nc.sync.dma_start(out=dbg_h[rows, :], in_=h_t[:])
                    nc.sync.dma_start(out=dbg_ht[t], in_=ht_s[:])
                    if t == 0:
                        nc.sync.dma_start(out=dbg_g[:], in_=g_t[:])

    nc.compile()
    return nc


_NC = None


def _get_nc():
    global _NC
    if _NC is None:
        _NC = build_nc()
    return _NC


def _in_maps(inputs):
    x32 = np.ascontiguousarray(np.asarray(inputs["x"], dtype=np.int32))
    emb = np.ascontiguousarray(np.asarray(inputs["emb"], dtype=np.float32))
    stat = {
        "emb": emb,
        "W1": np.ascontiguousarray(np.asarray(inputs["W1"], dtype=np.float32)),
        "b1": np.ascontiguousarray(np.asarray(inputs["b1"], dtype=np.float32)),
        "W2": np.ascontiguousarray(np.asarray(inputs["W2"], dtype=np.float32)),
        "b2": np.ascontiguousarray(np.asarray(inputs["b2"], dtype=np.float32)),
        "W3": np.ascontiguousarray(np.asarray(inputs["W3"], dtype=np.float32)),
        "b3": np.ascontiguousarray(np.asarray(inputs["b3"], dtype=np.float32)),
    }
    return [
        {"x": x32[c * B_CORE : (c + 1) * B_CORE], **stat} for c in range(NCORES)
    ]


def run(inputs, trace=False):
    """Run on 8 cores; returns (full output [16384,1] f32, BassKernelResults)."""
    from concourse.bass_utils import run_bass_kernel_spmd

    nc = _get_nc()
    res = run_bass_kernel_spmd(
        nc, _in_maps(inputs), core_ids=list(range(NCORES)), trace=trace
    )
    outs = [
        np.asarray(res.results[c]["out"], dtype=np.float32).reshape(B_CORE, 1)
        for c in range(NCORES)
    ]
    return np.concatenate(outs, axis=0), res


def kernel(**inputs) -> np.ndarray:
    out, _ = run(inputs, trace=False)
    return out
